# revision 14
# baseline (speedup 1.0000x reference)
"""PointTransformerConv layer on 8 trn2 NeuronCores.

Strategy (edge parallelism, dst-sharded):
- Nodes padded to NP (div by 1024). Core k owns dst range [k*NL, (k+1)*NL).
- Node-level linears are folded on host into two [36,64] matrices so the
  device computes per-node tables D = A@MD.T (U'=attn-dst part, r'=pos part)
  and Sn = A@MS.T (V=attn-src part, g=value part), A=[x|pos|1].
- Per edge: h = U'[dst]-V[src] (pre-BN attn feature), m = r'[dst]+g[src]
  (= v[src]+delta). Edges are gathered via dma_gather (int16 idx; Sn is
  split into 4 quartile subtables to fit int16).
- Pass 1 computes h,m for all edges, accumulates BN stats (sum h, sum h^2),
  stores h||m to a DRAM bounce. Stats are AllReduced across cores.
- Pass 2 reloads h||m, applies BN (folded scale/shift) + ReLU + second attn
  linear (block-diag matmul) + exp, forms y = ex*m, and scatter-adds
  [y||ex] into a per-node accumulator via dma_scatter_add.
  Softmax max-subtraction is dropped (mathematically exact shift-invariance;
  logits are BN-bounded so exp cannot overflow).
- dma_scatter_add loses updates when an index repeats within one call
  (HW RMW race, verified empirically), so the host orders each core's
  edges into rounds (k-th edge of each dst node): scatter calls never
  contain a duplicate dst. Dummy/padding tokens point at a scratch row.
- Final: out = (S/(den+eps)) @ Wu.T + bu + x on the core's node slice.
"""
import os
import sys

sys.path.insert(0, "/opt/trn_rl_repo")
sys.path.insert(0, "/opt/trn_rl_repo/concourse")

import numpy as np

import concourse.bacc as bacc
import concourse.bass as bass
import concourse.mybir as mybir
import concourse.tile as tile
from concourse.bass_utils import run_bass_kernel_spmd
from concourse.library_config import mlp as mlp_lib
from concourse.masks import make_identity

F32 = mybir.dt.float32
I16 = mybir.dt.int16
AF = mybir.ActivationFunctionType

EPS_BN = 1e-5
EPS_SM = 1e-16
NCORES = 8
GATHER_CALL = 4096   # tokens per dma_gather call
SCATTER_CALL = 2048  # max tokens per dma_scatter_add call
LAST_EXEC_NS = None


def _round_up(a, b):
    return (a + b - 1) // b * b


def _wrap16(idx):
    """[n] -> [128, n/16] int16; token i at [i%16, i//16], replicated x8."""
    w = idx.reshape(-1, 16).T
    return np.ascontiguousarray(np.tile(w, (8, 1))).astype(np.int16)


def _host_prep(x, pos, edge_index, W_src, W_dst, W_val, Wp, bp, Wa1, ba1,
               gamma, beta, Wa2, ba2, Wu, bu):
    N, C = x.shape
    E = edge_index.shape[1]
    NP = _round_up(N, 1024)
    NL = NP // 8
    NQ = NP // 4
    NQS = NQ + 32          # subtable rows incl zero/dummy rows
    assert NQ <= 32704

    # ---- weight folding ----
    Wa1Wd = (Wa1 @ W_dst).astype(np.float32)
    Wa1Ws = (Wa1 @ W_src).astype(np.float32)
    Wa1Wp = (Wa1 @ Wp).astype(np.float32)
    c1 = (Wa1 @ bp + ba1).astype(np.float32)
    MD = np.zeros((64, 36), np.float32)
    MD[0:32, 0:32] = Wa1Wd
    MD[0:32, 32:35] = Wa1Wp
    MD[0:32, 35] = c1
    MD[32:64, 32:35] = Wp
    MD[32:64, 35] = bp
    MS = np.zeros((64, 36), np.float32)
    MS[0:32, 0:32] = Wa1Ws
    MS[0:32, 32:35] = Wa1Wp
    MS[32:64, 0:32] = W_val
    MS[32:64, 32:35] = -Wp

    xpad = np.zeros((NP, C), np.float32)
    xpad[:N] = x
    pospad = np.zeros((NP, 3), np.float32)
    pospad[:N] = pos
    ones = np.zeros((NP,), np.float32)
    ones[:N] = 1.0
    A_T = np.concatenate([xpad.T, pospad.T, ones[None, :]], axis=0)  # [36, NP]
    A_T = np.ascontiguousarray(A_T)

    BD = np.zeros((128, 128), np.float32)
    for g in range(4):
        BD[32 * g:32 * g + 32, 32 * g:32 * g + 32] = Wa2.T
    ba2c4 = np.tile(ba2.astype(np.float32), 4).reshape(128, 1)
    I4 = np.zeros((32, 128), np.float32)
    for p in range(128):
        I4[p % 32, p] = 1.0
    burep = np.tile(bu.astype(np.float32).reshape(1, 32), (128, 1))

    # ---- edge sharding / ordering ----
    src = np.asarray(edge_index[0], np.int64)
    dst = np.asarray(edge_index[1], np.int64)
    core = dst // NL

    per_core = []  # (dstg, srcg, scg, qseg_bounds) raw (unpadded) per core
    counts = {}    # (q, r) -> list of per-core round sizes
    for k in range(NCORES):
        sel = np.nonzero(core == k)[0]
        d = dst[sel] - k * NL
        s = src[sel]
        q = s // NQ
        sl = s % NQ
        # rank of each edge within its (q, dst) group
        order = np.lexsort((d, q))
        d_o, q_o, sl_o = d[order], q[order], sl[order]
        # cumcount within (q, dst)
        key = q_o * NL + d_o
        uniq, start_idx = np.unique(key, return_index=True)
        rank = np.arange(len(key)) - np.repeat(start_idx, np.diff(
            np.append(start_idx, len(key))))
        rounds = {}
        for qq in range(4):
            mq = q_o == qq
            rk = rank[mq]
            for r in range(rk.max() + 1 if mq.any() else 0):
                mr = rk == r
                rounds[(qq, r)] = (d_o[mq][mr], sl_o[mq][mr])
                counts.setdefault((qq, r), [0] * NCORES)[k] = int(mr.sum())
        per_core.append(rounds)

    # aligned round sizes across cores (SPMD: same schedule everywhere)
    round_sizes = {}
    for (qq, r), cs in counts.items():
        round_sizes[(qq, r)] = _round_up(max(max(cs), 1), 128)

    # segment layout per quartile: rounds in order r=0,1,2,...
    seg_layout = {qq: sorted(r for (q2, r) in round_sizes if q2 == qq)
                  for qq in range(4)}
    seg_len = {qq: sum(round_sizes[(qq, r)] for r in seg_layout[qq])
               for qq in range(4)}
    E_pad = sum(seg_len.values())

    # schedule: gather windows (per quartile, <=GATHER_CALL, never cross
    # segment) and scatter pieces (within round AND window, <=SCATTER_CALL)
    gather_windows = []  # (qq, abs_start, length)
    scatter_pieces = []  # (abs_start, length)
    off = 0
    for qq in range(4):
        sl_ = seg_len[qq]
        p = 0
        while p < sl_:
            L = min(GATHER_CALL, sl_ - p)
            gather_windows.append((qq, off + p, L))
            p += L
        rp = off
        for r in seg_layout[qq]:
            n = round_sizes[(qq, r)]
            a = rp
            end = rp + n
            while a < end:
                nxt_win = (a - off) // GATHER_CALL * GATHER_CALL + GATHER_CALL + off
                b = min(a + SCATTER_CALL, end, nxt_win)
                scatter_pieces.append((a, b - a))
                a = b
            rp = end
        off += sl_

    # fill streams
    d_streams, s_streams, c_streams, x_slices, at_slices = [], [], [], [], []
    for k in range(NCORES):
        dstg = np.full(E_pad, NL, np.int64)       # dummy -> zero row of D
        srcg = np.full(E_pad, NQ, np.int64)       # dummy -> zero row of subtable
        scg = np.full(E_pad, NL, np.int64)        # dummy -> scratch acc row
        off = 0
        for qq in range(4):
            for r in seg_layout[qq]:
                n = round_sizes[(qq, r)]
                if (qq, r) in per_core[k]:
                    dd, ss = per_core[k][(qq, r)]
                    m = len(dd)
                    dstg[off:off + m] = dd
                    srcg[off:off + m] = ss
                    scg[off:off + m] = dd
                off += n
        d_streams.append(_wrap16(dstg))
        s_streams.append(_wrap16(srcg))
        c_streams.append(_wrap16(scg))
        x_slices.append(np.ascontiguousarray(xpad[k * NL:(k + 1) * NL]))
        at_slices.append(np.ascontiguousarray(A_T[:, k * NL:(k + 1) * NL]))

    shapes = dict(N=N, C=C, E=E, NP=NP, NL=NL, NQ=NQ, NQS=NQS, E_pad=E_pad,
                  gather_windows=gather_windows, scatter_pieces=scatter_pieces)
    consts = dict(A_T=A_T, MDT=np.ascontiguousarray(MD.T),
                  MST=np.ascontiguousarray(MS.T), BD=BD, ba2c4=ba2c4, I4=I4,
                  burep=burep, WuT=np.ascontiguousarray(Wu.T.astype(np.float32)),
                  gcol=gamma.astype(np.float32).reshape(32, 1),
                  bcol=beta.astype(np.float32).reshape(32, 1))
    streams = dict(d=d_streams, s=s_streams, c=c_streams, x=x_slices,
                   at=at_slices)
    return shapes, consts, streams


def _build(shapes):
    NP, NL, NQ, NQS = shapes["NP"], shapes["NL"], shapes["NQ"], shapes["NQS"]
    E_pad, E = shapes["E_pad"], shapes["E"]
    gw, sp = shapes["gather_windows"], shapes["scatter_pieces"]

    nc = bacc.Bacc("TRN2", debug=True)
    atf = nc.dram_tensor("atf", [36, NP], F32, kind="ExternalInput")
    ats = nc.dram_tensor("ats", [36, NL], F32, kind="ExternalInput")
    mdt = nc.dram_tensor("mdt", [36, 64], F32, kind="ExternalInput")
    mst = nc.dram_tensor("mst", [36, 64], F32, kind="ExternalInput")
    bd = nc.dram_tensor("bd", [128, 128], F32, kind="ExternalInput")
    ba2c4 = nc.dram_tensor("ba2c4", [128, 1], F32, kind="ExternalInput")
    i4 = nc.dram_tensor("i4", [32, 128], F32, kind="ExternalInput")
    burep = nc.dram_tensor("burep", [128, 32], F32, kind="ExternalInput")
    wut = nc.dram_tensor("wut", [32, 32], F32, kind="ExternalInput")
    gcol = nc.dram_tensor("gcol", [32, 1], F32, kind="ExternalInput")
    bcol = nc.dram_tensor("bcol", [32, 1], F32, kind="ExternalInput")
    xs = nc.dram_tensor("xs", [NL, 32], F32, kind="ExternalInput")
    dwi = nc.dram_tensor("dwi", [128, E_pad // 16], I16, kind="ExternalInput")
    swi = nc.dram_tensor("swi", [128, E_pad // 16], I16, kind="ExternalInput")
    sci = nc.dram_tensor("sci", [128, E_pad // 16], I16, kind="ExternalInput")

    outp = nc.dram_tensor("outp", [NL, 32], F32, kind="ExternalOutput")
    acc = nc.dram_tensor("acc", [NL + 128, 64], F32, kind="ExternalOutput")

    sn_tab = nc.dram_tensor("sn_tab", [4 * NQS, 64], F32)
    d_tab = nc.dram_tensor("d_tab", [NL + 128, 64], F32)
    bounce = nc.dram_tensor("bounce", [E_pad // 128, 128, 64], F32)

    with tile.TileContext(nc) as tc:
        nc.gpsimd.load_library(mlp_lib)
        import contextlib
        ctx = contextlib.ExitStack()
        cpool = ctx.enter_context(tc.tile_pool(name="consts", bufs=1))
        wpool = ctx.enter_context(tc.tile_pool(name="work", bufs=2))
        spool = ctx.enter_context(tc.tile_pool(name="streams", bufs=1))

        # consts
        mdt_t = cpool.tile([36, 64], F32)
        mst_t = cpool.tile([36, 64], F32)
        bd_t = cpool.tile([128, 128], F32)
        ba2_t = cpool.tile([128, 1], F32)
        i4_t = cpool.tile([32, 128], F32)
        burep_t = cpool.tile([128, 32], F32)
        wut_t = cpool.tile([32, 32], F32)
        gcol_t = cpool.tile([32, 1], F32)
        bcol_t = cpool.tile([32, 1], F32)
        ident = cpool.tile([128, 128], F32)
        ones_col = cpool.tile([128, 1], F32)
        zrow = cpool.tile([128, 64], F32)
        for t, d in [(mdt_t, mdt), (mst_t, mst), (bd_t, bd), (ba2_t, ba2c4),
                     (i4_t, i4), (burep_t, burep), (wut_t, wut),
                     (gcol_t, gcol), (bcol_t, bcol)]:
            nc.sync.dma_start(out=t[:], in_=d[:])
        make_identity(nc, ident[:])
        nc.vector.memset(ones_col[:], 1.0)
        nc.vector.memset(zrow[:], 0.0)

        # streams
        dwi_t = spool.tile([128, E_pad // 16], I16, tag="stream_a")
        swi_t = spool.tile([128, E_pad // 16], I16, tag="stream_b")
        nc.sync.dma_start(out=dwi_t[:], in_=dwi[:])
        nc.sync.dma_start(out=swi_t[:], in_=swi[:])

        # ---------- P0: node tables ----------
        p0pool = tc.tile_pool(name="psum0", bufs=2, space="PSUM")
        ppool = p0pool.__enter__()
        ntiles_q = NQ // 128  # tiles per quartile
        for qq in range(4):
            g = 0
            while g < ntiles_q:
                nb = min(4, ntiles_q - g)
                ps = ppool.tile([128, 256], F32, tag="p0")
                for j in range(nb):
                    t_global = qq * ntiles_q + g + j
                    at = wpool.tile([36, 128], F32, tag="at")
                    nc.sync.dma_start(out=at[:], in_=atf[:, t_global * 128:(t_global + 1) * 128])
                    nc.tensor.matmul(ps[:, 64 * j:64 * (j + 1)], lhsT=at[:],
                                     rhs=mst_t[:], start=True, stop=True)
                sb = wpool.tile([128, 4, 64], F32, tag="p0sb")
                nc.vector.tensor_copy(sb[:, 0:nb, :].rearrange("p a b -> p (a b)"),
                                      ps[:, 0:64 * nb])
                base = qq * NQS + g * 128
                nc.sync.dma_start(
                    out=sn_tab[base:base + nb * 128, :].rearrange("(a p) b -> p a b", p=128),
                    in_=sb[:, 0:nb, :])
                g += nb
            # zero rows of this subtable
            nc.sync.dma_start(out=sn_tab[qq * NQS + NQ:qq * NQS + NQ + 32, :],
                              in_=zrow[0:32, :])
        # D table (core's slice)
        ntiles_d = NL // 128
        g = 0
        while g < ntiles_d:
            nb = min(4, ntiles_d - g)
            ps = ppool.tile([128, 256], F32, tag="p0")
            for j in range(nb):
                at = wpool.tile([36, 128], F32, tag="at")
                nc.sync.dma_start(out=at[:], in_=ats[:, (g + j) * 128:(g + j + 1) * 128])
                nc.tensor.matmul(ps[:, 64 * j:64 * (j + 1)], lhsT=at[:],
                                 rhs=mdt_t[:], start=True, stop=True)
            sb = wpool.tile([128, 4, 64], F32, tag="p0sb")
            nc.vector.tensor_copy(sb[:, 0:nb, :].rearrange("p a b -> p (a b)"),
                                  ps[:, 0:64 * nb])
            nc.sync.dma_start(
                out=d_tab[g * 128:(g + nb) * 128, :].rearrange("(a p) b -> p a b", p=128),
                in_=sb[:, 0:nb, :])
            g += nb
        nc.sync.dma_start(out=d_tab[NL:NL + 128, :], in_=zrow[:])
        p0pool.__exit__(None, None, None)

        # ---------- P1: gather + h/m + stats + bounce ----------
        MAXG = GATHER_CALL // 128
        sacc = spool.tile([128, MAXG, 32], F32)
        qacc = spool.tile([128, MAXG, 32], F32)
        nc.vector.memset(sacc[:].rearrange("p a b -> p (a b)"), 0.0)
        nc.vector.memset(qacc[:].rearrange("p a b -> p (a b)"), 0.0)
        for (qq, wstart, L) in gw:
            ng = L // 128
            dt = wpool.tile([128, MAXG, 64], F32, tag="dt")
            st = wpool.tile([128, MAXG, 64], F32, tag="st")
            nc.gpsimd.dma_gather(
                out_ap=dt[:, 0:ng, :], in_ap=d_tab[:],
                idxs_ap=dwi_t[:, wstart // 16:(wstart + L) // 16],
                num_idxs=L, num_idxs_reg=L, elem_size=64, single_packet=False)
            nc.gpsimd.dma_gather(
                out_ap=st[:, 0:ng, :], in_ap=sn_tab[qq * NQS:(qq + 1) * NQS, :],
                idxs_ap=swi_t[:, wstart // 16:(wstart + L) // 16],
                num_idxs=L, num_idxs_reg=L, elem_size=64, single_packet=False)
            hm = wpool.tile([128, MAXG, 64], F32, tag="hm")
            nc.vector.tensor_sub(hm[:, 0:ng, 0:32], dt[:, 0:ng, 0:32], st[:, 0:ng, 0:32])
            nc.vector.tensor_add(hm[:, 0:ng, 32:64], dt[:, 0:ng, 32:64], st[:, 0:ng, 32:64])
            nc.vector.tensor_add(sacc[:, 0:ng, :], sacc[:, 0:ng, :], hm[:, 0:ng, 0:32])
            nc.vector.tensor_mul(dt[:, 0:ng, 0:32], hm[:, 0:ng, 0:32], hm[:, 0:ng, 0:32])
            nc.vector.tensor_add(qacc[:, 0:ng, :], qacc[:, 0:ng, :], dt[:, 0:ng, 0:32])
            nc.sync.dma_start(
                out=bounce[wstart // 128:(wstart + L) // 128, :, :].rearrange("a p b -> p a b"),
                in_=hm[:, 0:ng, :])

        # stats tree-fold over groups
        s_ = MAXG // 2
        while s_ >= 1:
            nc.vector.tensor_add(sacc[:, 0:s_, :], sacc[:, 0:s_, :], sacc[:, s_:2 * s_, :])
            nc.vector.tensor_add(qacc[:, 0:s_, :], qacc[:, 0:s_, :], qacc[:, s_:2 * s_, :])
            s_ //= 2
        stpool = tc.tile_pool(name="psumst", bufs=1, space="PSUM")
        ppool = stpool.__enter__()
        ps_s = ppool.tile([32, 1], F32, tag="stat_s")
        ps_q = ppool.tile([32, 1], F32, tag="stat_q")
        nc.tensor.matmul(ps_s[:], lhsT=sacc[:, 0, :], rhs=ones_col[:], start=True, stop=True)
        nc.tensor.matmul(ps_q[:], lhsT=qacc[:, 0, :], rhs=ones_col[:], start=True, stop=True)
        stat_sb = wpool.tile([32, 2], F32, tag="stat_sb")
        nc.vector.tensor_copy(stat_sb[:, 0:1], ps_s[:])
        nc.vector.tensor_copy(stat_sb[:, 1:2], ps_q[:])

        # AllReduce across cores (Tile-native collective via DRAM pool tiles)
        with tc.tile_pool(name="ccdram", bufs=1, space="DRAM") as ccdram:
            ccin_t = ccdram.tile([32, 2], F32)
            ccout_t = ccdram.tile([32, 2], F32)
            nc.gpsimd.dma_start(ccin_t[:], stat_sb[:])
            nc.gpsimd.collective_compute(
                "AllReduce", mybir.AluOpType.add,
                replica_groups=[list(range(NCORES))],
                ins=[ccin_t.opt()], outs=[ccout_t.opt()],
            )
            stat2 = wpool.tile([32, 2], F32, tag="stat2")
            nc.gpsimd.dma_start(stat2[:], ccout_t[:])

        # BN fold: k = gamma/sqrt(var+eps), b = beta - mu*k
        mu = wpool.tile([32, 1], F32)
        nc.vector.tensor_scalar_mul(mu[:], stat2[:, 0:1], 1.0 / E)
        ex2 = wpool.tile([32, 1], F32)
        nc.vector.tensor_scalar_mul(ex2[:], stat2[:, 1:2], 1.0 / E)
        musq = wpool.tile([32, 1], F32)
        nc.vector.tensor_mul(musq[:], mu[:], mu[:])
        var = wpool.tile([32, 1], F32)
        nc.vector.tensor_sub(var[:], ex2[:], musq[:])
        epsb = wpool.tile([32, 1], F32)
        nc.vector.memset(epsb[:], EPS_BN)
        nc.vector.tensor_add(var[:], var[:], epsb[:])
        sd = wpool.tile([32, 1], F32)
        nc.scalar.activation(sd[:], var[:], AF.Sqrt)
        istd = wpool.tile([32, 1], F32)
        nc.vector.reciprocal(istd[:], sd[:])
        kcol = wpool.tile([32, 1], F32)
        nc.vector.tensor_mul(kcol[:], gcol_t[:], istd[:])
        mk = wpool.tile([32, 1], F32)
        nc.vector.tensor_mul(mk[:], mu[:], kcol[:])
        bcol2 = wpool.tile([32, 1], F32)
        nc.vector.tensor_sub(bcol2[:], bcol_t[:], mk[:])
        ps_k = ppool.tile([128, 1], F32, tag="stat_s")
        ps_b = ppool.tile([128, 1], F32, tag="stat_q")
        nc.tensor.matmul(ps_k[:], lhsT=i4_t[:], rhs=kcol[:], start=True, stop=True)
        nc.tensor.matmul(ps_b[:], lhsT=i4_t[:], rhs=bcol2[:], start=True, stop=True)
        k4 = cpool.tile([128, 1], F32)
        b4 = cpool.tile([128, 1], F32)
        nc.vector.tensor_copy(k4[:], ps_k[:])
        nc.vector.tensor_copy(b4[:], ps_b[:])
        stpool.__exit__(None, None, None)

        # ---------- P2: BN/relu/linear/exp/y + scatter ----------
        sci_t = spool.tile([128, E_pad // 16], I16, tag="stream_a")
        nc.sync.dma_start(out=sci_t[:], in_=sci[:])
        p2pool = tc.tile_pool(name="psum2", bufs=2, space="PSUM")
        ppool = p2pool.__enter__()
        piece_iter = iter(sp)
        pending = next(piece_iter, None)
        for (qq, wstart, L) in gw:
            ng = L // 128
            hm = wpool.tile([128, MAXG, 64], F32, tag="dt")
            nc.sync.dma_start(out=hm[:, 0:ng, :],
                              in_=bounce[wstart // 128:(wstart + L) // 128, :, :]
                              .rearrange("a p b -> p a b"))
            ycat = wpool.tile([128, MAXG, 64], F32, tag="st")
            u = 0
            while u < ng:
                G = min(2, ng - u)
                hnT = wpool.tile([128, 128], F32, tag="hnT")
                for gi in range(G):
                    ps_h = ppool.tile([32, 128], F32, tag="pT")
                    nc.tensor.transpose(ps_h[:], in_=hm[:, u + gi, 0:32],
                                        identity=ident[:])
                    nc.scalar.activation(hnT[32 * gi:32 * (gi + 1), :], ps_h[:],
                                         AF.Relu, bias=b4[32 * gi:32 * gi + 32, :],
                                         scale=k4[32 * gi:32 * gi + 32, :])
                ps_l = ppool.tile([128, 128], F32, tag="pL")
                for gi in range(G):
                    nc.tensor.matmul(ps_l[32 * gi:32 * (gi + 1), :],
                                     lhsT=bd_t[32 * gi:32 * (gi + 1), 32 * gi:32 * (gi + 1)],
                                     rhs=hnT[32 * gi:32 * (gi + 1), :],
                                     start=True, stop=True)
                exT = wpool.tile([128, 128], F32, tag="exT")
                nc.scalar.activation(exT[0:32 * G, :], ps_l[0:32 * G, :], AF.Exp,
                                     bias=ba2_t[0:32 * G, :])
                ps_e = ppool.tile([128, 128], F32, tag="pE")
                nc.tensor.transpose(ps_e[:, 0:32 * G], in_=exT[0:32 * G, :],
                                    identity=ident[0:32 * G, 0:32 * G])
                nc.vector.tensor_mul(ycat[:, u:u + G, 0:32],
                                     ps_e[:, 0:32 * G].rearrange("p (a b) -> p a b", b=32),
                                     hm[:, u:u + G, 32:64])
                nc.vector.tensor_copy(ycat[:, u:u + G, 32:64],
                                      ps_e[:, 0:32 * G].rearrange("p (a b) -> p a b", b=32))
                u += G
            # scatter pieces inside this window
            while pending is not None and pending[0] < wstart + L:
                a, plen = pending
                ra = a - wstart
                nc.gpsimd.dma_scatter_add(
                    acc[:], ycat[:, ra // 128:(ra + plen) // 128, :],
                    sci_t[:, a // 16:(a + plen) // 16],
                    plen, plen, 64, single_packet=False)
                pending = next(piece_iter, None)

        p2pool.__exit__(None, None, None)
        # ---------- P3: final ----------
        p3pool = tc.tile_pool(name="psum3", bufs=2, space="PSUM")
        ppool = p3pool.__enter__()
        for t in range(NL // 128):
            ac = wpool.tile([128, 64], F32, tag="ac")
            nc.sync.dma_start(out=ac[:], in_=acc[t * 128:(t + 1) * 128, :])
            denp = wpool.tile([128, 32], F32, tag="denp")
            nc.vector.tensor_scalar_add(denp[:], ac[:, 32:64], EPS_SM)
            rec = wpool.tile([128, 32], F32, tag="rec")
            nc.vector.reciprocal(rec[:], denp[:])
            op = wpool.tile([128, 32], F32, tag="op")
            nc.vector.tensor_mul(op[:], ac[:, 0:32], rec[:])
            ps_t = ppool.tile([32, 128], F32, tag="pF")
            nc.tensor.transpose(ps_t[:], in_=op[:], identity=ident[:])
            opT = wpool.tile([32, 128], F32, tag="opT")
            nc.vector.tensor_copy(opT[:], ps_t[:])
            ps_o = ppool.tile([128, 32], F32, tag="pO")
            nc.tensor.matmul(ps_o[:], lhsT=opT[:], rhs=wut_t[:], start=True, stop=True)
            xt = wpool.tile([128, 32], F32, tag="xt")
            nc.sync.dma_start(out=xt[:], in_=xs[t * 128:(t + 1) * 128, :])
            o1 = wpool.tile([128, 32], F32, tag="o1")
            nc.vector.tensor_add(o1[:], ps_o[:], burep_t[:])
            nc.vector.tensor_add(o1[:], o1[:], xt[:])
            nc.sync.dma_start(out=outp[t * 128:(t + 1) * 128, :], in_=o1[:])
        p3pool.__exit__(None, None, None)
        ctx.close()

    nc.compile()
    return nc


def kernel(**inputs):
    shapes, consts, streams = _host_prep(**inputs)
    nc = _build(shapes)
    NL = shapes["NL"]
    in_maps = []
    for k in range(NCORES):
        in_maps.append({
            "atf": consts["A_T"], "ats": streams["at"][k],
            "mdt": consts["MDT"], "mst": consts["MST"], "bd": consts["BD"],
            "ba2c4": consts["ba2c4"], "i4": consts["I4"],
            "burep": consts["burep"], "wut": consts["WuT"],
            "gcol": consts["gcol"], "bcol": consts["bcol"],
            "xs": streams["x"][k], "dwi": streams["d"][k],
            "swi": streams["s"][k], "sci": streams["c"][k],
        })
    if os.environ.get("KERNEL_SIM"):
        from concourse import bass_interp
        sim = bass_interp.MultiCoreSim(nc, NCORES)
        for k in range(NCORES):
            for name, arr in in_maps[k].items():
                sim.cores[k].tensor(name)[:] = arr
            sim.cores[k].tensor("acc")[:] = 0
            sim.cores[k].tensor("outp")[:] = 0
        sim.simulate()
        outs = [np.array(sim.cores[k].mem_tensor("outp")).reshape(NL, 32)
                for k in range(NCORES)]
    else:
        global LAST_EXEC_NS
        import time as _time
        res = run_bass_kernel_spmd(nc, in_maps, list(range(NCORES)))
        if os.environ.get("KERNEL_TIME2"):
            # second execution reuses the compiled NEFF; wall time of this
            # call (incl. host<->device transfer) upper-bounds HW exec time
            t0 = _time.time()
            res = run_bass_kernel_spmd(nc, in_maps, list(range(NCORES)))
            LAST_EXEC_NS = int((_time.time() - t0) * 1e9)
        outs = [np.asarray(res.results[k]["outp"]).reshape(NL, 32)
                for k in range(NCORES)]
    full = np.concatenate(outs, axis=0)
    return full[:shapes["N"]].astype(np.float32)
